# revision 25
# baseline (speedup 1.0000x reference)
"""Trainium2 Bass kernel for the nn_Decoder LSTM-decoder problem.

Reference computation (per agent, 12 steps):
    gates = dec_in @ w_ih.T + h @ w_hh.T + (b_ih + b_hh)
    i, f, g, o = split(gates); c = sig(f)*c + sig(i)*tanh(g); h = sig(o)*tanh(c)
    rel = h @ w_hp.T + b_hp; dec_in = rel @ w_se.T + b_se
Output: rel per step, [12, N, 2].

Algebraic fusion: dec_in_t is linear in h_t, so for steps >= 2
    gates_t = h_{t-1} @ W_eff.T + b_eff,  W_eff = w_hh + w_ih @ w_se @ w_hp
and step 1 uses w_hh plus U = w_ih @ w_se applied to last_pos_rel.
last_pos is dead.

v2: the scalar (ACT) engine was the bottleneck (5 LUT passes/elem-step).
The three sigmoid gates are offloaded to a custom DVE op computing a
degree-5 odd minimax polynomial (max err 3.5e-3 on |z|<=3.6):
    sigma(z) ~= 0.5 + z'(1 + q1 t + q2 t^2),  z' = alpha*z, t = z'^2
with the leading coefficient normalized to 1.0 (ONE_F32 input) by scaling
the sigmoid-gate rows of W_eff (and biases) by alpha = c0 at fold time.
The op is 8 ALU stages: z=Src0+C0(bias); t=z*z; r=C2*t; s=r+C1; zt=z*t;
m=zt*s; y=z+m; out=y+Src1(0.5-tile). tanh stays on ACT (LUT-exact).
Cell-update elementwise muls/adds are split DVE/GPSIMD. Work assignment
per engine is tunable below. rel output is batched per step into one
[16,1024] PSUM tile -> one bias add -> 2 staging DMAs.

Distribution: pure data parallel, 8192 agents/core on 8 cores.
"""

import sys

if "/opt/trn_rl_repo" not in sys.path:
    sys.path.insert(0, "/opt/trn_rl_repo")

import numpy as np

T = 12          # steps
H = 128         # hidden dim
NCORES = 8
CH = 512        # agents per PSUM bank at fp32
PAIR = 2 * CH   # agents per gate-tile

# sigma deg-5 odd minimax on [-3.6, 3.6] (err 3.5e-3)
SIG_C = (0.24215996, -0.01431477, 0.0004515)
ALPHA = SIG_C[0]
Q1 = SIG_C[1] / ALPHA**3
Q2 = SIG_C[2] / ALPHA**5

# --- engine assignment tunables ------------------------------------------
# 4-char strings indexed by pair%4: a=ACT, d=DVE(custom), g=GPSIMD
SIG_ASSIGN = {"i": "dddd", "f": "aadd", "o": "aadd"}
TT_ASSIGN = {"m1": "gggg", "m2": "ggdd", "cadd": "gggd", "hmul": "dddd"}
REL_TRIO = True

_CACHE = {}


def _register_sig_op():
    import concourse.dve_ops as dve_ops
    from concourse.dve_ops import DveOp
    from concourse.dve_spec import Spec, Src0, Src1, C0, C1, C2, sq, lower
    from concourse.dve_uop import DveOpSpec

    for op in dve_ops.OPS:
        if op.name == "SIG_FULL_ANT":
            return op

    z = Src0 + C0
    t = sq(z)
    body = (z + (z * t) * (C1 + C2 * t)) + Src1

    def ref(in0, in1, c0, c1, c2):
        zz = in0.astype(np.float32) + c0
        tt = zz * zz
        return (zz + (zz * tt) * (c1 + c2 * tt)) + in1

    spec = Spec(body=body, reference=ref)
    probe = DveOpSpec(name="SIG_FULL_ANT", opcode=0,
                      uops=lower(spec, ver="v3"), rd1_en=True)
    op = DveOp("SIG_FULL_ANT", spec, subdim=False,
               uops_sha={"v3": probe.sha("v3")})
    dve_ops.OPS.append(op)
    dve_ops.CUSTOM_DVE_SPECS[op.name] = op.spec
    dve_ops._SUB_OPCODE_FOR_NAME[op.name] = (
        dve_ops._CUSTOM_DVE_ROW_BASE + len(dve_ops.OPS) - 1)
    assert dve_ops.get_dve_sub_opcode(op.name) < 0x20
    return op


def _build_program(npc):
    import concourse.bass as bass
    import concourse.tile as tile
    from concourse import bacc, mybir

    sig_op = _register_sig_op()

    dt = mybir.dt
    f32 = dt.float32
    bf16 = dt.bfloat16
    edt = bf16
    mdt = bf16
    Act = mybir.ActivationFunctionType
    Alu = mybir.AluOpType

    npair = npc // PAIR
    assert npc % PAIR == 0 and npc >= 2 * PAIR
    nblk = npc // 64   # output partition blocks (64 agents each)

    nc = bacc.Bacc(
        "TRN2",
        target_bir_lowering=False,
        debug=False,
        num_devices=NCORES,
    )

    def din(name, shape, dt_=None):
        return nc.dram_tensor(
            name, list(shape), dt_ or f32, kind="ExternalInput"
        ).ap()

    h0_d = din("h0", [npc, H])
    c0_d = din("c0", [npc, H])
    lpr_d = din("lpr", [npc, 2])
    # lhsT layouts, K on partitions. Gate order [i, f, o, g].
    wg_d = din("wg", [H, 4 * H], mdt)    # W_eff.T cols, sigma rows alpha-scaled
    whh_d = din("whh", [H, 4 * H], mdt)  # w_hh.T (step 1), same scaling
    u_d = din("u", [2, 4 * H], mdt)      # (w_ih @ w_se).T (step 1), scaled
    bias_d = din("bias", [H, 8])          # ACT bias: [b_eff | b1] x [i,f,o,g]
    biasa_d = din("biasa", [H, 8])        # alpha*bias for custom op, same cols
    whp_d = din("whp", [H, 2], mdt)      # w_hp.T
    bhpxy_d = din("bhpxy", [2, 1])        # rel bias [b_x, b_y]
    bhp66_d = din("bhp66", [66, 1])       # trio rel bias at rows 32k(+1)
    half_d = din("half", [128, PAIR], bf16)  # 0.5-filled Src1 tile
    ident_d = din("ident", [H, H])
    out_d = nc.dram_tensor("out", [T, npc, 2], f32, kind="ExternalOutput").ap()

    with tile.TileContext(nc) as tc:
        with (
            tc.tile_pool(name="wpool", bufs=1) as wp,
            tc.tile_pool(name="state", bufs=1) as state,
            tc.tile_pool(name="stage", bufs=4) as stage,
            tc.tile_pool(name="sig", bufs=3) as sigp,
            tc.tile_pool(name="tmp", bufs=3) as tmpp,
            tc.tile_pool(name="outp", bufs=2) as outp,
            tc.tile_pool(name="ps", bufs=3, space="PSUM") as psp,
            tc.tile_pool(name="psr", bufs=1, space="PSUM") as psr,
        ):
            def wtile(ap, shape, tag, dt_=None):
                t_ = wp.tile(list(shape), dt_ or f32, tag=tag)
                nc.sync.dma_start(t_[:], ap)
                return t_

            wg = wtile(wg_d, [H, 4 * H], "wg", mdt)
            whh = wtile(whh_d, [H, 4 * H], "whh", mdt)
            u = wtile(u_d, [2, 4 * H], "u", mdt)
            bias = wtile(bias_d, [H, 8], "bias")
            biasa = wtile(biasa_d, [H, 8], "biasa")
            whp = wtile(whp_d, [H, 2], "whp", mdt)
            bhpxy = wtile(bhpxy_d, [2, 1], "bhpxy")
            bhp66 = wtile(bhp66_d, [66, 1], "bhp66")
            half = wtile(half_d, [128, PAIR], "half", bf16)
            ident = wtile(ident_d, [H, H], "ident")

            h_sb = state.tile([H, npc], mdt, tag="h")
            c_sb = state.tile([H, npc], edt, tag="c")

            def pick(kind, p):
                a = SIG_ASSIGN[kind] if kind in SIG_ASSIGN else TT_ASSIGN[kind]
                return {"a": "act", "d": "dve", "g": "gp"}[a[p % 4]]

            def sig_eval(dst, gt, bcol, p, kind):
                """sigma of one alpha-scaled gate tile (PSUM) -> dst bf16."""
                eng = pick(kind, p)
                if eng == "act":
                    nc.scalar.activation(
                        dst[:], gt[:], Act.Sigmoid,
                        bias=bias[:, bcol:bcol + 1], scale=1.0 / ALPHA)
                else:
                    nc.vector._custom_dve(
                        sig_op, out=dst[:], in0=gt[:], in1=half[:],
                        s0=biasa[:, bcol:bcol + 1], s1=Q1, imm2=Q2)

            def tt(kind, p, out, a, b, add=False):
                eng = pick(kind, p)
                e = nc.gpsimd if eng == "gp" else nc.vector
                (e.tensor_add if add else e.tensor_mul)(out, a, b)

            def front(t, p, lpr_t):
                """Gates + sig_i/sig_f/tanh_g + m1/m2 + sig_o + c-add."""
                first = t == 0
                W = whh if first else wg
                bcol = 4 if first else 0
                cols = slice(p * PAIR, (p + 1) * PAIR)
                c_pr = c_sb[:, cols]
                gt = {}
                # wg column groups are [i, f, o, g] -> alloc i(0), f(1), g(3), o(2)
                for g in (0, 1, 3, 2):
                    gt[g] = psp.tile([128, 1024], f32, tag="ps", name=f"gt{g}")
                    wsl = slice(g * H, (g + 1) * H)
                    for half_i in range(2):
                        hs = slice((p * 2 + half_i) * CH,
                                   (p * 2 + half_i + 1) * CH)
                        osl = slice(half_i * CH, (half_i + 1) * CH)
                        if first:
                            nc.tensor.matmul(
                                gt[g][:, osl], u[:, wsl], lpr_t[:, osl],
                                start=True, stop=False)
                        nc.tensor.matmul(
                            gt[g][:, osl], W[:, wsl], h_sb[:, hs],
                            start=not first, stop=True)

                si = sigp.tile([128, PAIR], edt, tag="si")
                sf = sigp.tile([128, PAIR], edt, tag="sf")
                tg = sigp.tile([128, PAIR], edt, tag="tg")
                sig_eval(si, gt[0], bcol + 0, p, "i")
                sig_eval(sf, gt[1], bcol + 1, p, "f")
                nc.scalar.activation(tg[:], gt[3][:], Act.Tanh,
                                     bias=bias[:, bcol + 3:bcol + 4])
                m1 = tmpp.tile([128, PAIR], edt, tag="m1")
                tt("m1", p, m1[:], sf[:], c_pr)
                m2 = tmpp.tile([128, PAIR], edt, tag="m2")
                tt("m2", p, m2[:], si[:], tg[:])
                so = sigp.tile([128, PAIR], edt, tag="so")
                sig_eval(so, gt[2], bcol + 2, p, "o")
                tt("cadd", p, c_pr, m1[:], m2[:], add=True)
                return so

            def back(t, p, so):
                """tanh(c) + h update (deferred one unit)."""
                cols = slice(p * PAIR, (p + 1) * PAIR)
                h_pr = h_sb[:, cols]
                c_pr = c_sb[:, cols]
                tcl = sigp.tile([128, PAIR], edt, tag="tc")
                nc.scalar.activation(tcl[:], c_pr, Act.Tanh)
                tt("hmul", p, h_pr, so[:], tcl[:])

            blks = {}
            rel66 = None
            if REL_TRIO:
                rel66 = psr.tile([66, 1024], f32, tag="rel66")
                nc.vector.memset(rel66[:], 0.0)

            def rel_trio(entries):
                """rel = w_hp @ h for up to 3 pairs, packed at PSUM base
                partitions 0/32/64; one ACT Identity (+bias) drains all."""
                for k, (t, p) in enumerate(entries):
                    for half_i in range(2):
                        hs = slice((p * 2 + half_i) * CH,
                                   (p * 2 + half_i + 1) * CH)
                        osl = slice(half_i * CH, (half_i + 1) * CH)
                        nc.tensor.matmul(
                            rel66[32 * k:32 * k + 2, osl], whp[:],
                            h_sb[:, hs], start=True, stop=True)
                ex = tmpp.tile([66, 1024], bf16, tag="ex")
                nc.scalar.activation(ex[:], rel66[:], Act.Identity,
                                     bias=bhp66[:, 0:1])
                for k, (t, p) in enumerate(entries):
                    xblk, yblk = blks[t]
                    prt = slice(16 * p, 16 * (p + 1))
                    nc.sync.dma_start(xblk[prt, :],
                                      ex[32 * k:32 * k + 1, :])
                    nc.sync.dma_start(yblk[prt, :],
                                      ex[32 * k + 1:32 * k + 2, :])

            def rel_pair(t, p):
                """rel = w_hp @ h; bias + PSUM->SBUF via ACT Identity (bf16)."""
                xblk, yblk = blks[t]
                rp = psr.tile([2, 1024], f32, tag="rel")
                for half_i in range(2):
                    hs = slice((p * 2 + half_i) * CH,
                               (p * 2 + half_i + 1) * CH)
                    osl = slice(half_i * CH, (half_i + 1) * CH)
                    nc.tensor.matmul(
                        rp[0:2, osl], whp[:], h_sb[:, hs],
                        start=True, stop=True)
                ex = tmpp.tile([2, 1024], bf16, tag="ex")
                nc.scalar.activation(ex[:], rp[:], Act.Identity,
                                     bias=bhpxy[0:2, 0:1])
                prt = slice(16 * p, 16 * (p + 1))
                nc.sync.dma_start(xblk[prt, :], ex[0:1, :])
                nc.sync.dma_start(yblk[prt, :], ex[1:2, :])

            def flush_step(t):
                """Interleave x/y into the packed [64-agent, 128] layout."""
                xblk, yblk = blks.pop(t)
                relpk = outp.tile([nblk, 128], f32, tag="relpk")
                rv = relpk[:].rearrange("q (a k) -> q a k", k=2)
                nc.vector.tensor_copy(rv[:, :, 0], xblk[:])
                nc.vector.tensor_copy(rv[:, :, 1], yblk[:])
                nc.sync.dma_start(
                    out_d[t].rearrange("(q a) k -> q (a k)", a=64), relpk[:])

            def prologue_pair(p):
                cols = slice(p * PAIR, (p + 1) * PAIR)
                pt_h = psp.tile([128, 1024], f32, tag="ps")
                pt_c = psp.tile([128, 1024], f32, tag="ps")
                pt_l = psp.tile([128, 1024], f32, tag="ps")
                for j in range(8):
                    rows = slice(p * PAIR + j * 128, p * PAIR + (j + 1) * 128)
                    st = stage.tile([128, H], f32, tag="st_h")
                    nc.sync.dma_start(st[:], h0_d[rows, :])
                    nc.tensor.transpose(
                        pt_h[:, j * 128:(j + 1) * 128], st[:], ident[:])
                    st = stage.tile([128, H], f32, tag="st_c")
                    nc.sync.dma_start(st[:], c0_d[rows, :])
                    nc.tensor.transpose(
                        pt_c[:, j * 128:(j + 1) * 128], st[:], ident[:])
                    st = stage.tile([128, 2], f32, tag="st_l")
                    nc.sync.dma_start(st[:], lpr_d[rows, :])
                    nc.tensor.transpose(
                        pt_l[0:2, j * 128:(j + 1) * 128], st[:], ident[:])
                nc.vector.tensor_copy(h_sb[:, cols], pt_h[:])
                nc.vector.tensor_copy(c_sb[:, cols], pt_c[:])
                lpr_t = tmpp.tile([2, PAIR], mdt, tag="lprp", bufs=2)
                nc.vector.tensor_copy(lpr_t[:], pt_l[0:2, :])
                return lpr_t

            # ---- unit pipeline: FRONT(k) | BACK(k-1) | REL(k-2) ----
            units = [(t, p) for t in range(T) for p in range(npair)]
            pend_back = []
            pend_rel = []
            done_pairs = {t: 0 for t in range(T)}
            lpr_next = prologue_pair(0)

            def emit_rel(t, p):
                rel_pair(t, p)
                done_pairs[t] += 1
                if done_pairs[t] == npair:
                    flush_step(t)

            def emit_rel_trio(entries):
                rel_trio(entries)
                for (t, p) in entries:
                    done_pairs[t] += 1
                    if done_pairs[t] == npair:
                        flush_step(t)

            for k, (t, p) in enumerate(units):
                if t not in blks:
                    xb = outp.tile([nblk, 64], bf16, tag="xblk",
                                   name=f"xb{t}")
                    yb = outp.tile([nblk, 64], bf16, tag="yblk",
                                   name=f"yb{t}")
                    blks[t] = (xb, yb)
                lpr_t = None
                if t == 0:
                    lpr_t = lpr_next
                    if p + 1 < npair:
                        lpr_next = prologue_pair(p + 1)
                so = front(t, p, lpr_t)
                if pend_back:
                    back(*pend_back.pop(0))
                pend_back.append((t, p, so))
                if REL_TRIO:
                    if len(pend_rel) >= 4:
                        emit_rel_trio([pend_rel.pop(0) for _ in range(3)])
                else:
                    if len(pend_rel) >= min(3, npair):
                        emit_rel(*pend_rel.pop(0))
                pend_rel.append((t, p))
            while pend_back:
                back(*pend_back.pop(0))
            if REL_TRIO:
                while pend_rel:
                    emit_rel_trio([pend_rel.pop(0)
                                   for _ in range(min(3, len(pend_rel)))])
            else:
                while pend_rel:
                    emit_rel(*pend_rel.pop(0))

    nc.compile()
    return nc


def _fold_weights(w_ih, w_hh, b_ih, b_hh, w_se, b_se, w_hp, b_hp):
    """Host-side constant folding. Gate order [i, f, o, g] (torch order in
    the 4H rows is i, f, g, o). Sigma-gate rows (i, f, o) scaled by ALPHA."""
    perm = np.concatenate([
        np.arange(0, H), np.arange(H, 2 * H),
        np.arange(3 * H, 4 * H), np.arange(2 * H, 3 * H),
    ])
    W_eff = w_hh + w_ih @ w_se @ w_hp                      # [4H, H]
    b_eff = (b_hp @ w_se.T + b_se) @ w_ih.T + b_ih + b_hh  # [4H]
    U = w_ih @ w_se                                        # [4H, 2]
    b1 = b_se @ w_ih.T + b_ih + b_hh                       # [4H]

    Wp, bp = W_eff[perm], b_eff[perm]
    Whhp, Up, b1p = w_hh[perm], U[perm], b1[perm]

    # scale sigma-gate rows (first 3 blocks of the permuted [i, f, o, g])
    rowscale = np.ones((4 * H, 1), np.float32)
    rowscale[0:3 * H] = ALPHA
    Wp_s = Wp * rowscale
    Whhp_s = Whhp * rowscale
    Up_s = Up * rowscale

    f = np.float32
    import ml_dtypes
    mf = ml_dtypes.bfloat16
    bias = np.stack([bp[0:H], bp[H:2*H], bp[2*H:3*H], bp[3*H:4*H],
                     b1p[0:H], b1p[H:2*H], b1p[2*H:3*H], b1p[3*H:4*H]],
                    axis=1).astype(f)  # [H, 8] unscaled (ACT path)
    biasa = bias * ALPHA                # custom-op path (cols 3,7 unused)

    bhpxy = b_hp.astype(f).reshape(2, 1)
    bhp66 = np.zeros((66, 1), f)
    for k in range(3):
        bhp66[32 * k, 0] = b_hp[0]
        bhp66[32 * k + 1, 0] = b_hp[1]
    half = np.full((128, PAIR), 0.5, mf)
    return {
        "wg": np.ascontiguousarray(Wp_s.T.astype(mf)),
        "whh": np.ascontiguousarray(Whhp_s.T.astype(mf)),
        "u": np.ascontiguousarray(Up_s.T.astype(mf)),
        "bias": np.ascontiguousarray(bias, f),
        "biasa": np.ascontiguousarray(biasa, f),
        "whp": np.ascontiguousarray(w_hp.T.astype(mf)),
        "bhpxy": np.ascontiguousarray(bhpxy, f),
        "bhp66": np.ascontiguousarray(bhp66, f),
        "half": np.ascontiguousarray(half),
        "ident": np.eye(H, dtype=f),
    }


def kernel(last_pos, last_pos_rel, h0, c0,
           w_ih, w_hh, b_ih, b_hh, w_se, b_se, w_hp, b_hp):
    last_pos_rel = np.ascontiguousarray(np.asarray(last_pos_rel), np.float32)
    h0 = np.ascontiguousarray(np.asarray(h0), np.float32)
    c0 = np.ascontiguousarray(np.asarray(c0), np.float32)
    consts = _fold_weights(
        np.asarray(w_ih, np.float32), np.asarray(w_hh, np.float32),
        np.asarray(b_ih, np.float32), np.asarray(b_hh, np.float32),
        np.asarray(w_se, np.float32), np.asarray(b_se, np.float32),
        np.asarray(w_hp, np.float32), np.asarray(b_hp, np.float32),
    )

    npeds = h0.shape[0]
    npc = npeds // NCORES
    if "nc" not in _CACHE or _CACHE.get("npc") != npc:
        _CACHE["nc"] = _build_program(npc)
        _CACHE["npc"] = npc
    nc = _CACHE["nc"]

    in_maps = []
    for ci in range(NCORES):
        rows = slice(ci * npc, (ci + 1) * npc)
        m = {"h0": h0[rows], "c0": c0[rows], "lpr": last_pos_rel[rows]}
        m.update(consts)
        in_maps.append(m)

    from concourse.bass_utils import run_bass_kernel_spmd
    import os

    res = run_bass_kernel_spmd(
        nc, in_maps, list(range(NCORES)),
        tmpdir=os.environ.get("KERNEL_TRACE_DIR"),
    )
    _CACHE["exec_time_ns"] = res.exec_time_ns
    _CACHE["results"] = res
    outs = [np.asarray(res.results[i]["out"]) for i in range(NCORES)]
    return np.concatenate(outs, axis=1)
